# revision 25
# baseline (speedup 1.0000x reference)
"""Trainium2 Bass kernel for nn_CrossmotionModule (gnn_message_passing).

Reference computation (B=4, M=256, T=64, Dm=512, E=768):
    rel[b,m,t,n,k] = (c[b,m,t,k] - c[b,n,t,k]) * vis[b,m,t] * vis[b,n,t]
    fea[b,t,m,(n,k)] = rel                  # (B,T,M,512)
    h   = fea @ W1 + b1                     # (B,T,M,512)
    out = [h, pos] @ W2 + b2                # (B,T,M,768)

Algebraic collapse: with p = vis (B,T,M), u0 = p*c0, u1 = p*c1, the output is
a rank-4 outer product per (b,t) plus a constant:
    out[bt,m,e] = u0[m]*G0[e] + u1[m]*G1[e] - p[m]*G2[e] + const[m,e]
where, with the host-folded fused weight V2 = W1 @ W2[:512] (512, 768):
    G0[e] = sum_n p[n] V2[2n, e]
    G1[e] = sum_n p[n] V2[2n+1, e]
    G2[e] = sum_nk (p*c)[nk] V2[nk, e]
    const = b1 @ W2[:512] + pos @ W2[512:] + b2

v2 layout: G lives in SBUF with partition index (ck, r8, j) where the core's
32 bt-rows are split into 4 chunks (ck) of 8 (r8) and j indexes the 4 G rows
(G0, G1, G2, const). The prologue matmul produces exactly this layout (lhsT
columns ordered (ck, r8, j)), so there is NO reshuffle/DRAM bounce; const is
accumulated into the j=3 partitions by a K=1 matmul (host-provided mask row).
Main-loop matmuls for row r read G/U straight from partitions
p0 = 32*ck + 4*r8; rows from different ck chunks sit in different PE row
groups (explicit tile_position), so pairs of rows from different chunks run
concurrently on the 32-row-group sub-arrays. PE is warmed with dummy matmuls
during the input DMA so the HAM clock gate opens before real work.

Per row: 4 matmuls (m = 2p + w packing; N = 512/256/256/512) into one 3-bank
f32 PSUM tile, drained to bf16 out_sb by DVE+ACT with bank-aligned splits
alternating 1/2 vs 2/1 banks so the two engines never touch the same bank
concurrently. Per-row output DMA (393 KB) on the sync HWDGE queue; the whole
pipeline is paced by the ~35 us HBM-write floor of the 12.6 MB/core output.

Sharding: data-parallel over bt = (b,t) flattened; 256 rows / 8 cores = 32
rows per core. Weights replicated. No cross-device communication.
"""

import ml_dtypes
import numpy as np

B, M, T = 4, 256, 64
D_MOT, D_ABS, D_OUT = 512, 512, 768
N_CORES = 8
BT = B * T            # 256
R = BT // N_CORES     # 32 bt rows per core
E = D_OUT

BF16 = ml_dtypes.bfloat16

_CACHED_NC = {}


def _build_nc(fold_const):
    """Build the SPMD Bass program (identical for all 8 cores)."""
    import concourse.bacc as bacc
    import concourse.bass as bass
    import concourse.mybir as mybir
    import concourse.tile as tile

    f32 = mybir.dt.float32
    bf16 = mybir.dt.bfloat16
    PSUM = bass.MemorySpace.PSUM

    nc = bacc.Bacc("TRN2", target_bir_lowering=False, debug=False)

    # Per-core inputs (host-prepared layouts; see _prep_inputs).
    # lv: per k-chunk [la_k (128) | v2_k (768)] so one DMA unblocks matmul k.
    lv_d = nc.dram_tensor("lv", [128, 4 * 896], bf16, kind="ExternalInput")
    # ut is zero-padded to K=32 blocks: LDWEIGHTS partition starts must be
    # 32-aligned, so row r's stationary operand is a [32, 128] block at
    # partition 32*ck with zeros outside its own (r8, j) rows.
    ut_d = nc.dram_tensor("ut", [128, 2048], bf16, kind="ExternalInput")
    if fold_const:
        # [const_e (768) | j3 mask (128)]
        cst_d = nc.dram_tensor("cst", [1, 896], bf16, kind="ExternalInput")
    else:
        cst_d = nc.dram_tensor("cst", [128, 1536], f32, kind="ExternalInput")
    out_d = nc.dram_tensor("out", [R, M, E], bf16, kind="ExternalOutput")

    with tile.TileContext(nc) as tc:
        with tc.tile_pool(name="persist", bufs=1) as pers:
            lv_sb = pers.tile([128, 4 * 896], bf16)
            ut_sb = pers.tile([128, 2048], bf16)
            g_sb = pers.tile([128, 768], bf16)
            wt_sb = pers.tile([4, 640], bf16)
            scr_sb = pers.tile([1, 8], f32)
            if fold_const:
                cst_sb = pers.tile([1, 896], bf16)
            else:
                cst_sb = pers.tile([128, 1536], f32)

            # ---- prologue: G[(ck,r8,j), e] via the fused weight V2 ----
            with tc.tile_pool(name="prop", bufs=1, space=PSUM) as prop:
                # Input DMAs split across the two HWDGE queues.
                # gpsimd memsets first so the PE warm matmuls are unblocked
                # before the SWDGE descriptor generation for lv3/ut.
                nc.gpsimd.memset(scr_sb[:], 0.0)
                nc.gpsimd.memset(wt_sb[:], 1.0)
                nc.sync.dma_start(lv_sb[:, 0:896], lv_d[:, 0:896])
                nc.sync.dma_start(lv_sb[:, 896:1792], lv_d[:, 896:1792])
                nc.scalar.dma_start(cst_sb[:], cst_d[:])
                nc.scalar.dma_start(lv_sb[:, 1792:2688], lv_d[:, 1792:2688])
                nc.gpsimd.dma_start(lv_sb[:, 2688:3584], lv_d[:, 2688:3584])
                nc.gpsimd.dma_start(ut_sb[:], ut_d[:])
                # Warm the ACT activation table (after ACT's dma dispatches).
                nc.scalar.copy(scr_sb[0:1, 4:8], scr_sb[0:1, 0:4])
                # Dummy matmuls fill the PE until cst lands (HAM never
                # engages in this environment, so these are just filler).
                warm_ps = prop.tile([128, 512], f32)
                for _ in range(3):
                    nc.tensor.matmul(
                        warm_ps[:], wt_sb[0:4, 0:128], wt_sb[0:4, 128:640]
                    )

                g_ps0 = prop.tile([128, 512], f32)
                g_ps1 = prop.tile([128, 256], f32)
                if fold_const:
                    # const into the j=3 partitions first (K=1, lhsT = j3
                    # mask) — fills the PE while lv chunks are in flight.
                    mask = cst_sb[0:1, 768:896]
                    nc.tensor.matmul(
                        g_ps0[:], mask, cst_sb[0:1, 0:512], start=True, stop=False
                    )
                    nc.tensor.matmul(
                        g_ps1[:], mask, cst_sb[0:1, 512:768], start=True, stop=False
                    )
                # kc order matches expected chunk arrival (sync, scalar,
                # sync-2nd, gpsimd-after-gen); accumulation is commutative.
                kc_order = (0, 2, 1, 3)
                for i, kc in enumerate(kc_order):
                    la = lv_sb[:, kc * 896 : kc * 896 + 128]
                    v2 = lv_sb[:, kc * 896 + 128 : (kc + 1) * 896]
                    nc.tensor.matmul(
                        g_ps0[:], la, v2[:, 0:512],
                        start=(i == 0 and not fold_const), stop=(i == 3),
                    )
                    nc.tensor.matmul(
                        g_ps1[:], la, v2[:, 512:768],
                        start=(i == 0 and not fold_const), stop=(i == 3),
                    )
                nc.vector.tensor_copy(g_sb[:, 0:512], g_ps0[:])
                nc.scalar.copy(g_sb[:, 512:768], g_ps1[:, 0:256])

            # ---- main loop: out[r, m, e] = U4_r^T G4_r, rows paired across
            # ck chunks so their matmuls run in different PE row groups. ----
            with (
                tc.tile_pool(name="mp", bufs=8, space=PSUM) as mp,
                tc.tile_pool(name="op", bufs=6) as op,
            ):
                # Per-row matmuls: (psum bank idx, bank col range, g col
                # range, w half). Bank 1 is shared by the w0 tail and the
                # w1 head.
                mm_slices = (
                    (0, (0, 512), (0, 512), 0),
                    (1, (0, 256), (512, 768), 0),
                    (1, (256, 512), (0, 256), 1),
                    (2, (0, 512), (256, 768), 1),
                )
                # Pair p = (cc, r8): rows cc*16 + r8 and cc*16 + 8 + r8 go
                # out in one 786 KB DMA.
                out2_d = out_d.rearrange(
                    "(cc q r8) (p w) e -> cc r8 p q (w e)", cc=2, q=2, w=2
                )
                for s in range(16):
                    cka, ckb = (0, 1) if s < 8 else (2, 3)
                    r8 = s % 8
                    rows = (cka * 8 + r8, ckb * 8 + r8)
                    # Single-bank PSUM tiles so banks recycle at fine grain
                    # and consecutive pairs' matmuls pipeline.
                    ps = [[mp.tile([128, 512], f32, tag="ps", name=f"ps{r}_{b}")
                           for b in range(3)] for r in rows]
                    # Interleave the two rows' matmuls; different ck => PE
                    # row groups run them concurrently.
                    for b, (o_lo, o_hi), (g_lo, g_hi), w in mm_slices:
                        for k, ck in enumerate((cka, ckb)):
                            p0 = ck * 32
                            c0 = (r8 * 2 + w) * 128
                            nc.tensor.matmul(
                                ps[k][b][:, o_lo:o_hi],
                                ut_sb[p0 : p0 + 32, c0 : c0 + 128],
                                g_sb[p0 : p0 + 32, g_lo:g_hi],
                                tile_position=(p0, 0),
                            )
                    out_sb = op.tile([128, 3072], bf16, tag="out_sb",
                                     name=f"out_sb{rows[0]}")
                    for k, r in enumerate(rows):
                        half = out_sb[:, k * 1536 : (k + 1) * 1536]
                        # Per-bank drains, engines alternating so DVE and
                        # ACT never touch the same PSUM bank concurrently.
                        if fold_const:
                            eng = (nc.vector.tensor_copy, nc.scalar.copy)
                            for b in range(3):
                                eng[(b + k) % 2](
                                    half[:, b * 512 : (b + 1) * 512],
                                    ps[k][b][:],
                                )
                        else:
                            for b in range(3):
                                nc.vector.tensor_add(
                                    half[:, b * 512 : (b + 1) * 512],
                                    ps[k][b][:],
                                    cst_sb[:, b * 512 : (b + 1) * 512],
                                )
                    dst = out2_d[0 if s < 8 else 1, r8]  # [128, q=2, 1536]
                    src = out_sb.rearrange("p (q c) -> p q c", q=2)
                    if s == 0:
                        # Per-bank DMAs so the output stream starts as soon
                        # as the first bank of the first row is drained.
                        for q in range(2):
                            for b in range(3):
                                nc.sync.dma_start(
                                    dst[:, q, b * 512 : (b + 1) * 512],
                                    src[:, q, b * 512 : (b + 1) * 512],
                                )
                    elif s == 15:
                        # Split the last transfer so the tail is short.
                        nc.sync.dma_start(dst[:, 0], src[:, 0])
                        nc.sync.dma_start(dst[:, 1, 0:768], src[:, 1, 0:768])
                        nc.sync.dma_start(dst[:, 1, 768:1536], src[:, 1, 768:1536])
                    else:
                        nc.sync.dma_start(dst, src)
    nc.compile()
    return nc


def _prep_inputs(coords, mask, pos, w1, b1, w2, b2):
    """Host-side input sharding + weight-only constant folding."""
    nan0 = np.isnan(coords[..., 0])
    c = np.nan_to_num(coords)
    vis = np.where(nan0, np.float32(0.0), mask).astype(np.float32)

    p_all = np.ascontiguousarray(vis.transpose(0, 2, 1)).reshape(BT, M)
    c_bt = np.ascontiguousarray(c.transpose(0, 2, 1, 3)).reshape(BT, M, 2)
    q_all = (p_all[:, :, None] * c_bt).reshape(BT, 2 * M).astype(np.float32)

    W2t = w2[:D_MOT]
    W2b = w2[D_MOT:]
    const = (b1 @ W2t + b2)[None, :] + pos @ W2b          # (M, 768)
    fold_const = bool(np.all(const == const[0:1]))
    if fold_const:
        cst_dev = np.zeros((1, 896), dtype=BF16)
        cst_dev[0, 0:768] = const[0].astype(BF16)
        cst_dev[0, 768 + 3 : 896 : 4] = BF16(1.0)          # j==3 mask
    else:
        cst_dev = np.ascontiguousarray(
            const.astype(np.float32).reshape(128, 2, D_OUT)
        ).reshape(128, 1536)

    # Fused weight V2 = W1 @ W2t in bf16, 128 contraction rows per chunk.
    v2h = (w1 @ W2t).astype(np.float32).astype(BF16)      # (512, 768)
    v2h = v2h.reshape(4, 128, D_OUT)                      # (kc, kp, e)

    # U rows pair with G rows [G0; G1; G2; const]: [u0; u1; -p; ones].
    u0 = q_all[:, 0::2]
    u1 = q_all[:, 1::2]
    ones = np.ones_like(p_all)
    U4 = np.stack([u0, u1, -p_all, ones], axis=0)         # (4, BT, M)

    in_maps = []
    for i in range(N_CORES):
        rows = slice(i * R, (i + 1) * R)

        # la columns ordered (ck, r8, j): G row j of bt row ck*8+r8 at
        # partition 32*ck + 4*r8 + j. la rows = the 512 contraction dim.
        pc = p_all[rows].reshape(4, 8, M)                 # (ck, r8, n)
        qc = q_all[rows].reshape(4, 8, 2 * M)             # (ck, r8, k)
        la4 = np.zeros((2 * M, 4, 8, 4), np.float32)      # (k, ck, r8, j)
        la4[0::2, :, :, 0] = pc.transpose(2, 0, 1)
        la4[1::2, :, :, 1] = pc.transpose(2, 0, 1)
        la4[:, :, :, 2] = qc.transpose(2, 0, 1)
        la = la4.reshape(2 * M, 128).astype(BF16).reshape(4, 128, 128)
        # Interleave so lv_i[kp, kc*896 + c] = chunk kc, row kp, col c.
        lv_i = np.ascontiguousarray(
            np.concatenate([la, v2h], axis=2)             # (kc, kp, 128+768)
            .transpose(1, 0, 2)
        ).reshape(128, 4 * 896)

        # ut partitions (ck, r8', j); cols (r8, w, i) with m = 2i + w.
        # Zero except where r8' == r8 (K=32 alignment padding).
        uc = U4[:, rows].reshape(4, 4, 8, 128, 2)          # (j, ck, r8, i, w)
        ut8 = np.zeros((4, 8, 4, 8, 2, 128), np.float32)   # (ck,r8',j,r8,w,i)
        for r8 in range(8):
            ut8[:, r8, :, r8, :, :] = uc[:, :, r8].transpose(1, 0, 3, 2)
        ut_i = ut8.reshape(128, 2048).astype(BF16)

        in_maps.append({"lv": lv_i, "ut": ut_i, "cst": cst_dev})
    return in_maps, fold_const


def _run(inputs, trace=False, trace_kwargs=None):
    from concourse.bass_utils import run_bass_kernel_spmd

    coords = np.asarray(inputs["point_trajs_gt_coord"], dtype=np.float32)
    mask = np.asarray(inputs["point_trajs_visibility_mask"], dtype=np.float32)
    pos = np.asarray(inputs["pos_embed"], dtype=np.float32)
    w1 = np.asarray(inputs["fc1_w"], dtype=np.float32)
    b1 = np.asarray(inputs["fc1_b"], dtype=np.float32)
    w2 = np.asarray(inputs["fc_out_w"], dtype=np.float32)
    b2 = np.asarray(inputs["fc_out_b"], dtype=np.float32)

    in_maps, fold_const = _prep_inputs(coords, mask, pos, w1, b1, w2, b2)
    if fold_const not in _CACHED_NC:
        _CACHED_NC[fold_const] = _build_nc(fold_const)
    nc = _CACHED_NC[fold_const]

    res = run_bass_kernel_spmd(
        nc, in_maps, list(range(N_CORES)), trace=trace, **(trace_kwargs or {})
    )
    shards = [res.results[i]["out"] for i in range(N_CORES)]
    full = np.concatenate(shards, axis=0).reshape(B, T, M, D_OUT)
    return full.astype(np.float32), res


def kernel(**inputs):
    out, _ = _run(inputs, trace=False)
    return out


# revision 29
# speedup vs baseline: 1.0303x; 1.0303x over previous
"""Trainium2 Bass kernel for nn_CrossmotionModule (gnn_message_passing).

Reference computation (B=4, M=256, T=64, Dm=512, E=768):
    rel[b,m,t,n,k] = (c[b,m,t,k] - c[b,n,t,k]) * vis[b,m,t] * vis[b,n,t]
    fea[b,t,m,(n,k)] = rel                  # (B,T,M,512)
    h   = fea @ W1 + b1                     # (B,T,M,512)
    out = [h, pos] @ W2 + b2                # (B,T,M,768)

Algebraic collapse: with p = vis (B,T,M), u0 = p*c0, u1 = p*c1, the output is
a rank-4 outer product per (b,t) plus a constant:
    out[bt,m,e] = u0[m]*G0[e] + u1[m]*G1[e] - p[m]*G2[e] + const[m,e]
where, with the host-folded fused weight V2 = W1 @ W2[:512] (512, 768):
    G0[e] = sum_n p[n] V2[2n, e]
    G1[e] = sum_n p[n] V2[2n+1, e]
    G2[e] = sum_nk (p*c)[nk] V2[nk, e]
    const = b1 @ W2[:512] + pos @ W2[512:] + b2

Layout: G lives in SBUF with partition index (ck, r8, j) where the core's
32 bt-rows are split into 4 chunks (ck) of 8 (r8) and j indexes the 4 G rows
(G0, G1, G2, const). The prologue matmul produces exactly this layout (lhsT
columns ordered (ck, r8, j)), so there is NO reshuffle/DRAM bounce; const is
accumulated into the j=3 partitions by a K=1 matmul (host-provided mask row).
Main-loop matmul operands are zero-padded to K=32 blocks at partition 32*ck
(LDWEIGHTS partition starts must be 32-aligned; zero rows are free since
matmul cost is N-streaming only). Rows from different ck chunks sit in
different PE row groups (explicit tile_position), so pairs of rows from
chunks (0,1) / (2,3) run their matmuls concurrently on the 32-row-group
sub-arrays — the PE runs cold (1.2 GHz; the HAM clock gate never engages in
this environment), and 2-way concurrency keeps it off the critical path.

Per row: 4 matmuls (m = 2p + w packing; N = 512/256/256/512) into three
single-bank f32 PSUM tiles (bufs=8 so banks recycle at fine grain and
consecutive pairs pipeline), drained to bf16 out_sb by DVE+ACT alternating
per bank so the two engines never touch the same PSUM bank concurrently.
Output DMAs are 2-row 786 KB batches on the sync HWDGE queue (per-row for
the first pair, split tail for the last); the pipeline is paced by the
~33-35 us HBM-write floor of the 12.6 MB/core bf16 output. Input is spread
across the sync/scalar/gpsimd queues so the four lv chunk semaphores land
in consumption order.

Sharding: data-parallel over bt = (b,t) flattened; 256 rows / 8 cores = 32
rows per core. Weights replicated. No cross-device communication.
"""

import ml_dtypes
import numpy as np

B, M, T = 4, 256, 64
D_MOT, D_ABS, D_OUT = 512, 512, 768
N_CORES = 8
BT = B * T            # 256
R = BT // N_CORES     # 32 bt rows per core
E = D_OUT

BF16 = ml_dtypes.bfloat16

_CACHED_NC = {}


def _build_nc(fold_const):
    """Build the SPMD Bass program (identical for all 8 cores)."""
    import concourse.bacc as bacc
    import concourse.bass as bass
    import concourse.mybir as mybir
    import concourse.tile as tile

    f32 = mybir.dt.float32
    bf16 = mybir.dt.bfloat16
    PSUM = bass.MemorySpace.PSUM

    nc = bacc.Bacc("TRN2", target_bir_lowering=False, debug=False)

    # Per-core inputs (host-prepared layouts; see _prep_inputs).
    # lv: per k-chunk [la_k (128) | v2_k (768)] so one DMA unblocks matmul k.
    lv_d = nc.dram_tensor("lv", [128, 4 * 896], bf16, kind="ExternalInput")
    # ut is zero-padded to K=32 blocks: LDWEIGHTS partition starts must be
    # 32-aligned, so row r's stationary operand is a [32, 128] block at
    # partition 32*ck with zeros outside its own (r8, j) rows.
    ut_d = nc.dram_tensor("ut", [128, 2048], bf16, kind="ExternalInput")
    if fold_const:
        # [const_e (768) | j3 mask (128)]
        cst_d = nc.dram_tensor("cst", [1, 896], bf16, kind="ExternalInput")
    else:
        cst_d = nc.dram_tensor("cst", [128, 1536], f32, kind="ExternalInput")
    out_d = nc.dram_tensor("out", [R, M, E], bf16, kind="ExternalOutput")

    with tile.TileContext(nc) as tc:
        with tc.tile_pool(name="persist", bufs=1) as pers:
            lv_sb = pers.tile([128, 4 * 896], bf16)
            ut_sb = pers.tile([128, 2048], bf16)
            g_sb = pers.tile([128, 768], bf16)
            wt_sb = pers.tile([4, 640], bf16)
            scr_sb = pers.tile([1, 8], f32)
            if fold_const:
                cst_sb = pers.tile([1, 896], bf16)
            else:
                cst_sb = pers.tile([128, 1536], f32)

            # ---- prologue: G[(ck,r8,j), e] via the fused weight V2 ----
            with tc.tile_pool(name="prop", bufs=1, space=PSUM) as prop:
                # Input DMAs split across the two HWDGE queues.
                # gpsimd memsets first so the PE warm matmuls are unblocked
                # before the SWDGE descriptor generation for lv3/ut.
                nc.gpsimd.memset(scr_sb[:], 0.0)
                nc.gpsimd.memset(wt_sb[:], 1.0)
                nc.sync.dma_start(lv_sb[:, 0:896], lv_d[:, 0:896])
                nc.sync.dma_start(lv_sb[:, 896:1792], lv_d[:, 896:1792])
                nc.scalar.dma_start(lv_sb[:, 1792:2688], lv_d[:, 1792:2688])
                nc.scalar.dma_start(cst_sb[:], cst_d[:])
                nc.gpsimd.dma_start(lv_sb[:, 2688:3584], lv_d[:, 2688:3584])
                nc.gpsimd.dma_start(ut_sb[:], ut_d[:])
                # Warm the ACT activation table (after ACT's dma dispatches).
                nc.scalar.copy(scr_sb[0:1, 4:8], scr_sb[0:1, 0:4])
                # Dummy matmuls fill the PE until the first lv chunk lands
                # (HAM never engages in this environment; just filler).
                warm_ps = prop.tile([128, 512], f32)
                for _ in range(5):
                    nc.tensor.matmul(
                        warm_ps[:], wt_sb[0:4, 0:128], wt_sb[0:4, 128:640]
                    )

                g_ps0 = prop.tile([128, 512], f32)
                g_ps1 = prop.tile([128, 256], f32)
                for kc in range(4):
                    la = lv_sb[:, kc * 896 : kc * 896 + 128]
                    v2 = lv_sb[:, kc * 896 + 128 : (kc + 1) * 896]
                    nc.tensor.matmul(
                        g_ps0[:], la, v2[:, 0:512],
                        start=(kc == 0), stop=(kc == 3 and not fold_const),
                    )
                    nc.tensor.matmul(
                        g_ps1[:], la, v2[:, 512:768],
                        start=(kc == 0), stop=(kc == 3 and not fold_const),
                    )
                if fold_const:
                    # const into the j=3 partitions: K=1, lhsT = j3 mask.
                    mask = cst_sb[0:1, 768:896]
                    nc.tensor.matmul(
                        g_ps0[:], mask, cst_sb[0:1, 0:512], start=False, stop=True
                    )
                    nc.tensor.matmul(
                        g_ps1[:], mask, cst_sb[0:1, 512:768], start=False, stop=True
                    )
                nc.vector.tensor_copy(g_sb[:, 0:512], g_ps0[:])
                nc.scalar.copy(g_sb[:, 512:768], g_ps1[:, 0:256])

            # ---- main loop: out[r, m, e] = U4_r^T G4_r, rows paired across
            # ck chunks so their matmuls run in different PE row groups. ----
            with (
                tc.tile_pool(name="mp", bufs=8, space=PSUM) as mp,
                tc.tile_pool(name="op", bufs=6) as op,
            ):
                # Per-row matmuls: (psum bank idx, bank col range, g col
                # range, w half). Bank 1 is shared by the w0 tail and the
                # w1 head.
                mm_slices = (
                    (0, (0, 512), (0, 512), 0),
                    (1, (0, 256), (512, 768), 0),
                    (1, (256, 512), (0, 256), 1),
                    (2, (0, 512), (256, 768), 1),
                )
                # Pair p = (cc, r8): rows cc*16 + r8 and cc*16 + 8 + r8 go
                # out in one 786 KB DMA.
                out2_d = out_d.rearrange(
                    "(cc q r8) (p w) e -> cc r8 p q (w e)", cc=2, q=2, w=2
                )
                for s in range(16):
                    cka, ckb = (0, 1) if s < 8 else (2, 3)
                    r8 = s % 8
                    rows = (cka * 8 + r8, ckb * 8 + r8)
                    # Single-bank PSUM tiles so banks recycle at fine grain
                    # and consecutive pairs' matmuls pipeline.
                    ps = [[mp.tile([128, 512], f32, tag="ps", name=f"ps{r}_{b}")
                           for b in range(3)] for r in rows]
                    # Interleave the two rows' matmuls; different ck => PE
                    # row groups run them concurrently.
                    for b, (o_lo, o_hi), (g_lo, g_hi), w in mm_slices:
                        for k, ck in enumerate((cka, ckb)):
                            p0 = ck * 32
                            c0 = (r8 * 2 + w) * 128
                            nc.tensor.matmul(
                                ps[k][b][:, o_lo:o_hi],
                                ut_sb[p0 : p0 + 32, c0 : c0 + 128],
                                g_sb[p0 : p0 + 32, g_lo:g_hi],
                                tile_position=(p0, 0),
                            )
                    out_sb = op.tile([128, 3072], bf16, tag="out_sb",
                                     name=f"out_sb{rows[0]}")
                    for k, r in enumerate(rows):
                        half = out_sb[:, k * 1536 : (k + 1) * 1536]
                        # Per-bank drains, engines alternating so DVE and
                        # ACT never touch the same PSUM bank concurrently.
                        if fold_const:
                            eng = (nc.vector.tensor_copy, nc.scalar.copy)
                            for b in range(3):
                                eng[(b + k) % 2](
                                    half[:, b * 512 : (b + 1) * 512],
                                    ps[k][b][:],
                                )
                        else:
                            for b in range(3):
                                nc.vector.tensor_add(
                                    half[:, b * 512 : (b + 1) * 512],
                                    ps[k][b][:],
                                    cst_sb[:, b * 512 : (b + 1) * 512],
                                )
                    dst = out2_d[0 if s < 8 else 1, r8]  # [128, q=2, 1536]
                    src = out_sb.rearrange("p (q c) -> p q c", q=2)
                    if s == 0:
                        # Per-row DMAs so the output stream starts as soon
                        # as the first row is drained.
                        nc.sync.dma_start(dst[:, 0], src[:, 0])
                        nc.sync.dma_start(dst[:, 1], src[:, 1])
                    elif s == 15:
                        # Split the last transfer so the tail is short.
                        nc.sync.dma_start(dst[:, 0], src[:, 0])
                        nc.sync.dma_start(dst[:, 1, 0:768], src[:, 1, 0:768])
                        nc.sync.dma_start(dst[:, 1, 768:1536], src[:, 1, 768:1536])
                    else:
                        nc.sync.dma_start(dst, src)
    nc.compile()
    return nc


def _prep_inputs(coords, mask, pos, w1, b1, w2, b2):
    """Host-side input sharding + weight-only constant folding."""
    nan0 = np.isnan(coords[..., 0])
    c = np.nan_to_num(coords)
    vis = np.where(nan0, np.float32(0.0), mask).astype(np.float32)

    p_all = np.ascontiguousarray(vis.transpose(0, 2, 1)).reshape(BT, M)
    c_bt = np.ascontiguousarray(c.transpose(0, 2, 1, 3)).reshape(BT, M, 2)
    q_all = (p_all[:, :, None] * c_bt).reshape(BT, 2 * M).astype(np.float32)

    W2t = w2[:D_MOT]
    W2b = w2[D_MOT:]
    const = (b1 @ W2t + b2)[None, :] + pos @ W2b          # (M, 768)
    fold_const = bool(np.all(const == const[0:1]))
    if fold_const:
        cst_dev = np.zeros((1, 896), dtype=BF16)
        cst_dev[0, 0:768] = const[0].astype(BF16)
        cst_dev[0, 768 + 3 : 896 : 4] = BF16(1.0)          # j==3 mask
    else:
        cst_dev = np.ascontiguousarray(
            const.astype(np.float32).reshape(128, 2, D_OUT)
        ).reshape(128, 1536)

    # Fused weight V2 = W1 @ W2t in bf16, 128 contraction rows per chunk.
    v2h = (w1 @ W2t).astype(np.float32).astype(BF16)      # (512, 768)
    v2h = v2h.reshape(4, 128, D_OUT)                      # (kc, kp, e)

    # U rows pair with G rows [G0; G1; G2; const]: [u0; u1; -p; ones].
    u0 = q_all[:, 0::2]
    u1 = q_all[:, 1::2]
    ones = np.ones_like(p_all)
    U4 = np.stack([u0, u1, -p_all, ones], axis=0)         # (4, BT, M)

    in_maps = []
    for i in range(N_CORES):
        rows = slice(i * R, (i + 1) * R)

        # la columns ordered (ck, r8, j): G row j of bt row ck*8+r8 at
        # partition 32*ck + 4*r8 + j. la rows = the 512 contraction dim.
        pc = p_all[rows].reshape(4, 8, M)                 # (ck, r8, n)
        qc = q_all[rows].reshape(4, 8, 2 * M)             # (ck, r8, k)
        la4 = np.zeros((2 * M, 4, 8, 4), np.float32)      # (k, ck, r8, j)
        la4[0::2, :, :, 0] = pc.transpose(2, 0, 1)
        la4[1::2, :, :, 1] = pc.transpose(2, 0, 1)
        la4[:, :, :, 2] = qc.transpose(2, 0, 1)
        la = la4.reshape(2 * M, 128).astype(BF16).reshape(4, 128, 128)
        # Interleave so lv_i[kp, kc*896 + c] = chunk kc, row kp, col c.
        lv_i = np.ascontiguousarray(
            np.concatenate([la, v2h], axis=2)             # (kc, kp, 128+768)
            .transpose(1, 0, 2)
        ).reshape(128, 4 * 896)

        # ut partitions (ck, r8', j); cols (r8, w, i) with m = 2i + w.
        # Zero except where r8' == r8 (K=32 alignment padding).
        uc = U4[:, rows].reshape(4, 4, 8, 128, 2)          # (j, ck, r8, i, w)
        ut8 = np.zeros((4, 8, 4, 8, 2, 128), np.float32)   # (ck,r8',j,r8,w,i)
        for r8 in range(8):
            ut8[:, r8, :, r8, :, :] = uc[:, :, r8].transpose(1, 0, 3, 2)
        ut_i = ut8.reshape(128, 2048).astype(BF16)

        in_maps.append({"lv": lv_i, "ut": ut_i, "cst": cst_dev})
    return in_maps, fold_const


def _run(inputs, trace=False, trace_kwargs=None):
    from concourse.bass_utils import run_bass_kernel_spmd

    coords = np.asarray(inputs["point_trajs_gt_coord"], dtype=np.float32)
    mask = np.asarray(inputs["point_trajs_visibility_mask"], dtype=np.float32)
    pos = np.asarray(inputs["pos_embed"], dtype=np.float32)
    w1 = np.asarray(inputs["fc1_w"], dtype=np.float32)
    b1 = np.asarray(inputs["fc1_b"], dtype=np.float32)
    w2 = np.asarray(inputs["fc_out_w"], dtype=np.float32)
    b2 = np.asarray(inputs["fc_out_b"], dtype=np.float32)

    in_maps, fold_const = _prep_inputs(coords, mask, pos, w1, b1, w2, b2)
    if fold_const not in _CACHED_NC:
        _CACHED_NC[fold_const] = _build_nc(fold_const)
    nc = _CACHED_NC[fold_const]

    res = run_bass_kernel_spmd(
        nc, in_maps, list(range(N_CORES)), trace=trace, **(trace_kwargs or {})
    )
    shards = [res.results[i]["out"] for i in range(N_CORES)]
    full = np.concatenate(shards, axis=0).reshape(B, T, M, D_OUT)
    return full.astype(np.float32), res


def kernel(**inputs):
    out, _ = _run(inputs, trace=False)
    return out
